# revision 3
# baseline (speedup 1.0000x reference)
"""BoundaryLoss kernel v2 for 8 Trainium2 NeuronCores.

Math (equivalent to the reference):
  boundary(i,j) = [L(i,j+1) != L(i,j-1)] OR [L(i+1,j) != L(i-1,j)]  (indices
    clamped at the image edge, matching np.gradient's one-sided edge diffs)
  ce = ln(e^x0 + e^x1 + e^x2) - x[label]
  loss = sum(ce * boundary) / (sum(boundary) + 1e-8)

Design notes (HW-measured costs; one full-core pass = 18432 free elems):
  - preds ship fp16 (halves HBM traffic; |x|<6 so fp16 rounding is ~1e-5
    on the final loss). Labels ship int16, VALUE-ENCODED as
    {0:0x0000, 1:0x0100, 2:0x0101} so the two selection copy_predicated
    masks come free: the full int16 is nonzero iff L!=0 (CP1), the low
    byte is nonzero iff L==2 (CP2, via a bitcast stride-2 int8 view).
  - Two label layouts ship: edge-padded per-partition windows [P, 8, 770]
    (each partition's 6 rows + halo rows, one contiguous DMA; boundary
    compares are in-partition shifted APs, no seams/slivers), and a flat
    contiguous copy for the CP predicates (DMA bytes are cheaper than the
    staging copy the strided window view would force on the DVE).
  - DVE rates: 2-byte tensor_tensor 2x (~10.8us/pass), copy_predicated 1x
    (~20.4us). The DVE stream is only: 2 adds, nqx, bnd, CP1, CP2, ce.
  - GpSimd (Pool) takes nqy and w = ce*bnd off the DVE's back.
  - PE accumulates sum(bnd), sum(w) via ones-matmuls over flat 512-col
    slabs into two PSUM accumulators; host does the final f64 division.
  - ACT: one merged exp over [P, 3, 2304] per half-image, one Ln over
    [P, 4608] per image (ACT cost is 1568ns/inst + 0.833ns/elem, so few
    big instructions).
  - Software pipelining: image i's ce/w (which need image i's Ln) are
    emitted inside image i+1's DVE/Pool streams so no engine head-blocks.
"""

import numpy as np

B, C, H, W = 32, 3, 768, 768
NCORES = 8
BLOC = B // NCORES      # images per core
P = 128
TPB = H // P            # rows per partition (6)
NH = 2                  # chunks (half-images) per image
RPC = TPB // NH         # rows per chunk (3)
CHW = RPC * W           # 2304
WIN = TPB + 2           # label window rows per partition (8)
WP = W + 2              # padded row length (770)

_CACHE = {}

# int16 label encoding: full word nonzero iff L!=0; low byte nonzero iff L==2
ENC = np.array([0x0000, 0x0100, 0x0101], dtype=np.int16)


def _build():
    import concourse.bacc as bacc
    import concourse.tile as tile
    import concourse.mybir as mybir

    fp32 = mybir.dt.float32
    fp16 = mybir.dt.float16
    bf16 = mybir.dt.bfloat16
    i16 = mybir.dt.int16
    i8 = mybir.dt.int8
    Alu = mybir.AluOpType
    Act = mybir.ActivationFunctionType

    nc = bacc.Bacc(
        "TRN2",
        target_bir_lowering=False,
        debug=False,
        enable_asserts=False,
        num_devices=NCORES,
    )
    preds = nc.dram_tensor(
        "preds", [BLOC, P, C, TPB * W], fp16, kind="ExternalInput"
    ).ap()
    labs = nc.dram_tensor(
        "labs", [BLOC, P, WIN, WP], i16, kind="ExternalInput"
    ).ap()
    lval = nc.dram_tensor(
        "lval", [BLOC, P, TPB * W], i16, kind="ExternalInput"
    ).ap()
    idin = nc.dram_tensor("idin", [P, P], bf16, kind="ExternalInput").ap()
    outp = nc.dram_tensor("partials", [1, 1024], fp32, kind="ExternalOutput").ap()

    SLABS = [(0, 512), (512, 1024), (1024, 1536), (1536, 2048), (2048, 2304)]
    NMM = BLOC * NH * len(SLABS)  # accumulated matmuls per PSUM tile

    with tile.TileContext(nc) as tc:
        with (
            tc.tile_pool(name="ps", bufs=1, space="PSUM") as ps_pool,
            tc.tile_pool(name="lab", bufs=2) as lab_pool,
            tc.tile_pool(name="lv", bufs=2) as lv_pool,
            tc.tile_pool(name="xin", bufs=3) as x_pool,
            tc.tile_pool(name="eact", bufs=2) as e_pool,
            tc.tile_pool(name="lse", bufs=2) as l_pool,
            tc.tile_pool(name="nq", bufs=2) as nq_pool,
            tc.tile_pool(name="bnd", bufs=4) as bnd_pool,
            tc.tile_pool(name="cw", bufs=2) as cw_pool,
            tc.tile_pool(name="acc", bufs=1) as acc_pool,
        ):
            ones = acc_pool.tile([P, 1], bf16, name="ones")
            nc.vector.memset(ones[:], 1.0)
            iden = acc_pool.tile([P, P], bf16, name="iden")
            nc.sync.dma_start(out=iden[:], in_=idin[:, :])
            pb = ps_pool.tile([1, 512], fp32, name="pb")
            pcb = ps_pool.tile([1, 512], fp32, name="pcb")
            s2p = ps_pool.tile([P, CHW], fp32, name="s2p")

            nb_mm = [0]
            nw_mm = [0]

            def colsum(psacc, src_flat, counter):
                for (a0, a1) in SLABS:
                    k = counter[0]
                    nc.tensor.matmul(
                        psacc[:, 0 : a1 - a0],
                        ones[:],
                        src_flat[:, a0:a1],
                        start=(k == 0),
                        stop=(k == NMM - 1),
                    )
                    counter[0] += 1

            pending = []  # deferred (lse_slice, xsel, bnd) per chunk

            def flush_pending():
                while pending:
                    lseh, xsel, bnd = pending.pop(0)
                    ce = cw_pool.tile([P, CHW], fp16, name="ce", tag="ce")
                    nc.vector.tensor_sub(ce[:], lseh, xsel)
                    w = cw_pool.tile([P, CHW], bf16, name="w", tag="w")
                    nc.vector.tensor_mul(w[:], ce[:], bnd[:])
                    colsum(pcb, w[:], nw_mm)

            for b in range(BLOC):
                T = lab_pool.tile([P, WIN, WP], i16, name="T", tag="T")
                nc.sync.dma_start(out=T[:], in_=labs[b])
                lv = lv_pool.tile([P, TPB * W], i16, name="lv", tag="lv")
                nc.sync.dma_start(out=lv[:], in_=lval[b])
                lv8 = lv[:].bitcast(i8).rearrange("p (n two) -> p n two", two=2)
                lse = l_pool.tile([P, TPB * W], fp16, name="lse", tag="lse")
                for h in range(NH):
                    x = x_pool.tile([P, C, CHW], fp16, name=f"x{h}", tag="x")
                    nc.sync.dma_start(
                        out=x[:],
                        in_=preds[b, :, :, h * CHW : (h + 1) * CHW],
                    )
                    e = e_pool.tile([P, C, CHW], bf16, name=f"e{h}", tag="e")
                    nc.scalar.activation(e[:], x[:], Act.Exp)
                    # s2 = e0+e1+e2 on the PE: per 512-slab, 3 identity
                    # matmuls accumulate the channel sum into PSUM
                    for (a0, a1) in SLABS:
                        for c in range(C):
                            nc.tensor.matmul(
                                s2p[:, a0:a1],
                                iden[:],
                                e[:, c, a0:a1],
                                start=(c == 0),
                                stop=(c == C - 1),
                            )
                    nc.scalar.activation(
                        lse[:, h * CHW : (h + 1) * CHW], s2p[:], Act.Ln
                    )
                    # boundary compares on the label window (row r = image
                    # row r-1; this chunk's rows start at window row 3h+1)
                    r0 = h * RPC + 1
                    Ll = T[:, r0 : r0 + RPC, 0:W]
                    Lr = T[:, r0 : r0 + RPC, 2 : 2 + W]
                    Lu = T[:, r0 - 1 : r0 - 1 + RPC, 1 : 1 + W]
                    Ld = T[:, r0 + 1 : r0 + 1 + RPC, 1 : 1 + W]
                    nqx = nq_pool.tile([P, RPC, W], i16, name="nqx", tag="nqx")
                    nc.vector.tensor_tensor(nqx[:], Ll, Lr, Alu.not_equal)
                    nqy = nq_pool.tile([P, RPC, W], i16, name="nqy", tag="nqy")
                    nc.vector.tensor_tensor(nqy[:], Lu, Ld, Alu.not_equal)
                    bnd = bnd_pool.tile([P, CHW], bf16, name=f"bnd{h}", tag="bnd")
                    nc.vector.tensor_tensor(
                        bnd[:], nqx[:].rearrange("p a b -> p (a b)"),
                        nqy[:].rearrange("p a b -> p (a b)"), Alu.max
                    )
                    # selection: overwrite x0 with x1 where L!=0, then x2
                    # where L==2 (ordered CPs; predicates straight from lval)
                    lvh = lv[:, h * CHW : (h + 1) * CHW]
                    m2h = lv8[:, h * CHW : (h + 1) * CHW, 0:1].squeeze(2)
                    nc.vector.copy_predicated(x[:, 0], lvh, x[:, 1])
                    nc.vector.copy_predicated(x[:, 0], m2h, x[:, 2])
                    colsum(pb, bnd[:], nb_mm)
                    # one-chunk software pipelining: emit the PREVIOUS
                    # chunk's ce/w here so the DVE never stalls on this
                    # chunk's Ln
                    flush_pending()
                    pending.append(
                        (lse[:, h * CHW : (h + 1) * CHW], x[:, 0], bnd)
                    )
            flush_pending()

            sb = acc_pool.tile([1, 1024], fp32, name="sb")
            nc.vector.tensor_copy(sb[:, 0:512], pb[:, :])
            nc.vector.tensor_copy(sb[:, 512:1024], pcb[:, :])
            nc.sync.dma_start(out=outp[:, :], in_=sb[:])

    # Pin Exp/Ln to the one ACT table set containing both so the table loads
    # once instead of thrashing between sets.
    from concourse import hw_specs

    KEEP = "natural_log_exp_and_others"
    orig = hw_specs.get_activation_tables

    def only_combined(arch):
        t = orig(arch)
        return {name: (funcs if name == KEEP else set()) for name, funcs in t.items()}

    patched = []
    for mod in (hw_specs, bacc):
        if getattr(mod, "get_activation_tables", None) is not None:
            patched.append((mod, mod.get_activation_tables))
            mod.get_activation_tables = only_combined
    try:
        nc.compile()
    finally:
        for mod, fn in patched:
            mod.get_activation_tables = fn
    return nc


def _get_nc():
    if "nc" not in _CACHE:
        _CACHE["nc"] = _build()
    return _CACHE["nc"]


def prep_inputs(predictions, labels):
    """Host-side sharding/layout prep. Returns per-core input maps."""
    preds = (
        np.ascontiguousarray(predictions)
        .astype(np.float16)
        .reshape(NCORES, BLOC, C, P, TPB * W)
        .transpose(0, 1, 3, 2, 4)  # -> [core, img, P, C, 4608]
    )
    preds = np.ascontiguousarray(preds)
    lab = ENC[np.asarray(labels).astype(np.int64)]  # [B, 768, 768] i16 encoded
    lp = np.pad(lab, ((0, 0), (1, 1), (1, 1)), mode="edge")  # [B, 770, 770]
    ridx = (TPB * np.arange(P))[:, None] + np.arange(WIN)[None, :]  # [P, 8]
    lw = np.ascontiguousarray(lp[:, ridx, :]).reshape(NCORES, BLOC, P, WIN, WP)
    lv = np.ascontiguousarray(lab).reshape(NCORES, BLOC, P, TPB * W)
    iden = np.eye(P, dtype=np.float32)
    import ml_dtypes
    iden = iden.astype(ml_dtypes.bfloat16)
    return [
        {"preds": preds[i], "labs": lw[i], "lval": lv[i], "idin": iden}
        for i in range(NCORES)
    ]


def finish(results):
    """Combine per-core [1, 1024] partials into the scalar loss."""
    tot_b = 0.0
    tot_cb = 0.0
    for r in results:
        p = r["partials"].astype(np.float64)
        tot_b += p[0, :512].sum()
        tot_cb += p[0, 512:].sum()
    return np.float32(tot_cb / (tot_b + 1e-8))


def kernel(predictions, labels):
    from concourse.bass_utils import run_bass_kernel_spmd

    nc = _get_nc()
    in_maps = prep_inputs(predictions, labels)
    res = run_bass_kernel_spmd(nc, in_maps, list(range(NCORES))).results
    return finish(res)


# revision 4
# speedup vs baseline: 1.0132x; 1.0132x over previous
"""BoundaryLoss kernel v2 for 8 Trainium2 NeuronCores.

Math (equivalent to the reference):
  boundary(i,j) = [L(i,j+1) != L(i,j-1)] OR [L(i+1,j) != L(i-1,j)]  (indices
    clamped at the image edge, matching np.gradient's one-sided edge diffs)
  ce = ln(e^x0 + e^x1 + e^x2) - x[label]
  loss = sum(ce * boundary) / (sum(boundary) + 1e-8)

Design notes (HW-measured costs; one full-core pass = 18432 free elems):
  - preds ship fp16 (halves HBM traffic; |x|<6 so fp16 rounding is ~1e-5
    on the final loss). Labels ship int16, VALUE-ENCODED as
    {0:0x0000, 1:0x0100, 2:0x0101} so the two selection copy_predicated
    masks come free: the full int16 is nonzero iff L!=0 (CP1), the low
    byte is nonzero iff L==2 (CP2, via a bitcast stride-2 int8 view).
  - Two label layouts ship: edge-padded per-partition windows [P, 8, 770]
    (each partition's 6 rows + halo rows, one contiguous DMA; boundary
    compares are in-partition shifted APs, no seams/slivers), and a flat
    contiguous copy for the CP predicates (DMA bytes are cheaper than the
    staging copy the strided window view would force on the DVE).
  - DVE rates: 2-byte tensor_tensor 2x (~10.8us/pass), copy_predicated 1x
    (~20.4us). The DVE stream per chunk is only: nqx, nqy, bnd-max, CP1,
    CP2, ce, w.  (GpSimd tensor ops compile but return wrong results on
    HW via this toolchain, so everything elementwise stays on the DVE.)
  - The channel sum s2 = e0+e1+e2 runs on the otherwise-idle PE: per
    512-col slab, 3 identity-weight matmuls accumulate into a PSUM tile,
    and Ln reads straight from PSUM.  This moved two full tensor_add
    passes off the DVE (the bottleneck engine).
  - PE also accumulates sum(bnd), sum(ce*bnd) via ones-matmuls into two
    [1,512] PSUM accumulators; host does the final f64 sum + division.
  - ACT: one merged exp over [P, 3, 2304] per half-image, one Ln per
    chunk (ACT cost ~1.6us/inst fixed + 0.833ns/elem, so few big
    instructions).
  - One-chunk software pipelining: chunk k's ce/w (which need chunk k's
    Ln) are emitted after chunk k+1's CPs so the DVE never stalls.
"""

import numpy as np

B, C, H, W = 32, 3, 768, 768
NCORES = 8
BLOC = B // NCORES      # images per core
P = 128
TPB = H // P            # rows per partition (6)
NH = 2                  # chunks (half-images) per image
RPC = TPB // NH         # rows per chunk (3)
CHW = RPC * W           # 2304
WIN = TPB + 2           # label window rows per partition (8)
WP = W + 2              # padded row length (770)

_CACHE = {}

# int16 label encoding: full word nonzero iff L!=0; low byte nonzero iff L==2
ENC = np.array([0x0000, 0x0100, 0x0101], dtype=np.int16)


def _build():
    import concourse.bacc as bacc
    import concourse.tile as tile
    import concourse.mybir as mybir

    fp32 = mybir.dt.float32
    fp16 = mybir.dt.float16
    bf16 = mybir.dt.bfloat16
    i16 = mybir.dt.int16
    i8 = mybir.dt.int8
    Alu = mybir.AluOpType
    Act = mybir.ActivationFunctionType

    nc = bacc.Bacc(
        "TRN2",
        target_bir_lowering=False,
        debug=False,
        enable_asserts=False,
        num_devices=NCORES,
    )
    preds = nc.dram_tensor(
        "preds", [BLOC, P, C, TPB * W], fp16, kind="ExternalInput"
    ).ap()
    labs = nc.dram_tensor(
        "labs", [BLOC, P, WIN, WP], i16, kind="ExternalInput"
    ).ap()
    lval = nc.dram_tensor(
        "lval", [BLOC, P, TPB * W], i16, kind="ExternalInput"
    ).ap()
    idin = nc.dram_tensor("idin", [P, P], bf16, kind="ExternalInput").ap()
    outp = nc.dram_tensor("partials", [1, 1024], fp32, kind="ExternalOutput").ap()

    SLABS = [(0, 512), (512, 1024), (1024, 1536), (1536, 2048), (2048, 2304)]
    NMM = BLOC * NH * len(SLABS)  # accumulated matmuls per PSUM tile

    with tile.TileContext(nc) as tc:
        with (
            tc.tile_pool(name="ps", bufs=1, space="PSUM") as ps_pool,
            tc.tile_pool(name="lab", bufs=2) as lab_pool,
            tc.tile_pool(name="lv", bufs=2) as lv_pool,
            tc.tile_pool(name="xin", bufs=3) as x_pool,
            tc.tile_pool(name="eact", bufs=2) as e_pool,
            tc.tile_pool(name="lse", bufs=2) as l_pool,
            tc.tile_pool(name="nq", bufs=2) as nq_pool,
            tc.tile_pool(name="bnd", bufs=4) as bnd_pool,
            tc.tile_pool(name="cw", bufs=2) as cw_pool,
            tc.tile_pool(name="acc", bufs=1) as acc_pool,
        ):
            ones = acc_pool.tile([P, 1], bf16, name="ones")
            nc.vector.memset(ones[:], 1.0)
            iden = acc_pool.tile([P, P], bf16, name="iden")
            nc.sync.dma_start(out=iden[:], in_=idin[:, :])
            pb = ps_pool.tile([1, 512], fp32, name="pb")
            pcb = ps_pool.tile([1, 512], fp32, name="pcb")
            s2p = ps_pool.tile([P, CHW], fp32, name="s2p")

            nb_mm = [0]
            nw_mm = [0]

            def colsum(psacc, src_flat, counter):
                for (a0, a1) in SLABS:
                    k = counter[0]
                    nc.tensor.matmul(
                        psacc[:, 0 : a1 - a0],
                        ones[:],
                        src_flat[:, a0:a1],
                        start=(k == 0),
                        stop=(k == NMM - 1),
                    )
                    counter[0] += 1

            pending = []  # deferred (lse_slice, xsel, bnd) per chunk

            def flush_pending():
                while pending:
                    lseh, xsel, bnd = pending.pop(0)
                    ce = cw_pool.tile([P, CHW], fp16, name="ce", tag="ce")
                    nc.vector.tensor_sub(ce[:], lseh, xsel)
                    w = cw_pool.tile([P, CHW], bf16, name="w", tag="w")
                    nc.vector.tensor_mul(w[:], ce[:], bnd[:])
                    colsum(pcb, w[:], nw_mm)

            for b in range(BLOC):
                T = lab_pool.tile([P, WIN, WP], i16, name="T", tag="T")
                nc.sync.dma_start(out=T[:], in_=labs[b])
                lv = lv_pool.tile([P, TPB * W], i16, name="lv", tag="lv")
                nc.sync.dma_start(out=lv[:], in_=lval[b])
                lv8 = lv[:].bitcast(i8).rearrange("p (n two) -> p n two", two=2)
                lse = l_pool.tile([P, TPB * W], fp16, name="lse", tag="lse")
                for h in range(NH):
                    x = x_pool.tile([P, C, CHW], fp16, name=f"x{h}", tag="x")
                    nc.sync.dma_start(
                        out=x[:],
                        in_=preds[b, :, :, h * CHW : (h + 1) * CHW],
                    )
                    e = e_pool.tile([P, C, CHW], bf16, name=f"e{h}", tag="e")
                    nc.scalar.activation(e[:], x[:], Act.Exp)
                    # s2 = e0+e1+e2 on the PE: per 512-slab, 3 identity
                    # matmuls accumulate the channel sum into PSUM
                    for (a0, a1) in SLABS:
                        for c in range(C):
                            nc.tensor.matmul(
                                s2p[:, a0:a1],
                                iden[:],
                                e[:, c, a0:a1],
                                start=(c == 0),
                                stop=(c == C - 1),
                            )
                    nc.scalar.activation(
                        lse[:, h * CHW : (h + 1) * CHW], s2p[:], Act.Ln
                    )
                    # boundary compares on the label window (row r = image
                    # row r-1; this chunk's rows start at window row 3h+1)
                    r0 = h * RPC + 1
                    Ll = T[:, r0 : r0 + RPC, 0:W]
                    Lr = T[:, r0 : r0 + RPC, 2 : 2 + W]
                    Lu = T[:, r0 - 1 : r0 - 1 + RPC, 1 : 1 + W]
                    Ld = T[:, r0 + 1 : r0 + 1 + RPC, 1 : 1 + W]
                    nqx = nq_pool.tile([P, RPC, W], i16, name="nqx", tag="nqx")
                    nc.vector.tensor_tensor(nqx[:], Ll, Lr, Alu.not_equal)
                    nqy = nq_pool.tile([P, RPC, W], i16, name="nqy", tag="nqy")
                    nc.vector.tensor_tensor(nqy[:], Lu, Ld, Alu.not_equal)
                    bnd = bnd_pool.tile([P, CHW], bf16, name=f"bnd{h}", tag="bnd")
                    nc.vector.tensor_tensor(
                        bnd[:], nqx[:].rearrange("p a b -> p (a b)"),
                        nqy[:].rearrange("p a b -> p (a b)"), Alu.max
                    )
                    # selection: overwrite x0 with x1 where L!=0, then x2
                    # where L==2 (ordered CPs; predicates straight from lval)
                    lvh = lv[:, h * CHW : (h + 1) * CHW]
                    m2h = lv8[:, h * CHW : (h + 1) * CHW, 0:1].squeeze(2)
                    nc.vector.copy_predicated(x[:, 0], lvh, x[:, 1])
                    nc.vector.copy_predicated(x[:, 0], m2h, x[:, 2])
                    colsum(pb, bnd[:], nb_mm)
                    # one-chunk software pipelining: emit the PREVIOUS
                    # chunk's ce/w here so the DVE never stalls on this
                    # chunk's Ln
                    flush_pending()
                    pending.append(
                        (lse[:, h * CHW : (h + 1) * CHW], x[:, 0], bnd)
                    )
            flush_pending()

            sb = acc_pool.tile([1, 1024], fp32, name="sb")
            nc.vector.tensor_copy(sb[:, 0:512], pb[:, :])
            nc.vector.tensor_copy(sb[:, 512:1024], pcb[:, :])
            nc.sync.dma_start(out=outp[:, :], in_=sb[:])

    # Pin Exp/Ln to the one ACT table set containing both so the table loads
    # once instead of thrashing between sets.
    from concourse import hw_specs

    KEEP = "natural_log_exp_and_others"
    orig = hw_specs.get_activation_tables

    def only_combined(arch):
        t = orig(arch)
        return {name: (funcs if name == KEEP else set()) for name, funcs in t.items()}

    patched = []
    for mod in (hw_specs, bacc):
        if getattr(mod, "get_activation_tables", None) is not None:
            patched.append((mod, mod.get_activation_tables))
            mod.get_activation_tables = only_combined
    try:
        nc.compile()
    finally:
        for mod, fn in patched:
            mod.get_activation_tables = fn
    return nc


def _get_nc():
    if "nc" not in _CACHE:
        _CACHE["nc"] = _build()
    return _CACHE["nc"]


def prep_inputs(predictions, labels):
    """Host-side sharding/layout prep. Returns per-core input maps."""
    preds = (
        np.ascontiguousarray(predictions)
        .astype(np.float16)
        .reshape(NCORES, BLOC, C, P, TPB * W)
        .transpose(0, 1, 3, 2, 4)  # -> [core, img, P, C, 4608]
    )
    preds = np.ascontiguousarray(preds)
    lab = ENC[np.asarray(labels).astype(np.int64)]  # [B, 768, 768] i16 encoded
    lp = np.pad(lab, ((0, 0), (1, 1), (1, 1)), mode="edge")  # [B, 770, 770]
    ridx = (TPB * np.arange(P))[:, None] + np.arange(WIN)[None, :]  # [P, 8]
    lw = np.ascontiguousarray(lp[:, ridx, :]).reshape(NCORES, BLOC, P, WIN, WP)
    lv = np.ascontiguousarray(lab).reshape(NCORES, BLOC, P, TPB * W)
    iden = np.eye(P, dtype=np.float32)
    import ml_dtypes
    iden = iden.astype(ml_dtypes.bfloat16)
    return [
        {"preds": preds[i], "labs": lw[i], "lval": lv[i], "idin": iden}
        for i in range(NCORES)
    ]


def finish(results):
    """Combine per-core [1, 1024] partials into the scalar loss."""
    tot_b = 0.0
    tot_cb = 0.0
    for r in results:
        p = r["partials"].astype(np.float64)
        tot_b += p[0, :512].sum()
        tot_cb += p[0, 512:].sum()
    return np.float32(tot_cb / (tot_b + 1e-8))


def kernel(predictions, labels):
    from concourse.bass_utils import run_bass_kernel_spmd

    nc = _get_nc()
    in_maps = prep_inputs(predictions, labels)
    res = run_bass_kernel_spmd(nc, in_maps, list(range(NCORES))).results
    return finish(res)
